# revision 29
# baseline (speedup 1.0000x reference)
"""Distributed Bass attention kernel for 8 TRN2 NeuronCores.

Problem: nn_Attention (B=2, NQ=512, NCTX=16384, QDIM=CDIM=512, H=8, D=64).

Sharding: data parallel on batch (2) x tensor parallel on heads (4 groups of
2 heads) = 8 cores. Core i handles batch i//4, heads [2*(i%4), 2*(i%4)+1].
Each core computes its head-slice of the attention output, normalizes it,
multiplies by its slice of Wout to produce a PARTIAL output projection
[out 512, q 512]; the host sums the 4 partials per batch (no on-device
collective) and adds bout.

Device-side layout/perf choices:
  - context is passed pre-transposed AND pre-cast to bf16 ([d_model, seq]),
    halving HBM traffic for the big tensor (16 MiB/core). wk/wv stay bf16
    (fp8 weights cost ~3% rel err - measured, too much).
  - scores are computed transposed (simT[j, i]) in bf16 (contraction d=64),
    so the context mask is a per-partition bias of the exp and the softmax
    denominator falls out of the AV matmul via a ones-column in V.
  - exp runs on BOTH the Scalar engine (ACT exp -> fp8e4) and the Vector
    engine (Schraudolph affine+clamp to uint8; the bit-pattern IS the fp8),
    split 4:3 to balance engines; the Schraudolph min-op runs on GpSimd.
  - attention weights (pt) and V are fp8e4; pt is scaled 1/16 (softmax
    ratio invariant) so exp never overflows fp8 range.
  - AV runs as fp8 DoubleRow over j-tile pairs (contraction 256/instr,
    2x fewer PE rows than bf16).
  - attention j-tile pairs interleave with the next context chunk's KV
    units so the PE never idles; KT copies ride ACT, V copies ride DVE.
"""
import sys

sys.path.insert(0, '/opt/trn_rl_repo')

import numpy as np
import ml_dtypes

import concourse.bacc as bacc
import concourse.mybir as mybir
import concourse.tile as tile
from concourse.bass_utils import run_bass_kernel_spmd

F32 = mybir.dt.float32
BF16 = mybir.dt.bfloat16
FP8 = mybir.dt.float8e4
U8 = mybir.dt.uint8
AF = mybir.ActivationFunctionType
ALU = mybir.AluOpType
DR = mybir.MatmulPerfMode.DoubleRow

B = 2
NQ = 512          # query tokens (i)
NCTX = 16384      # context tokens (j)
DM = 512          # model dim
HEADS = 8
DH = 64
INNER = 512
N_CORES = 8

KC = 4              # d_model chunks of 128
NJT = NCTX // 128   # 128 j-tiles
NPAIR = NJT // 2    # 64 j-tile pairs
JCH = 2048          # context j-chunk per DMA (1 MiB fp8 source)
NCH = NCTX // JCH

SCALE = DH ** -0.5                       # s_true = psum * SCALE
PT_SHIFT = -2.7726                       # pt = exp(s_true)/16, keeps fp8<240
MASK_BIG = 30000.0
# Schraudolph constants for fp8e4 (ieee e4m3, bias 7, 3 mantissa bits):
#   u8 = clamp(A*psum + B[j], 0, 119); bitpattern u8 == fp8 ~ exp-approx
EXP_A = float(8.0 * np.log2(np.e) * SCALE)
EXP_B = float(56.0 + 8.0 * PT_SHIFT * np.log2(np.e) - 0.44)


def build_nc():
    nc = bacc.Bacc(None, target_bir_lowering=False, debug=False, num_devices=N_CORES)

    xt_d = nc.dram_tensor("xT", [128, KC * NQ], BF16, kind="ExternalInput")
    ctx_d = nc.dram_tensor("ctxb", [DM, NCTX], BF16, kind="ExternalInput")
    msk_d = nc.dram_tensor("maskt", [128, NJT], U8, kind="ExternalInput")
    wq_d = nc.dram_tensor("wq", [128, KC * 128], BF16, kind="ExternalInput")
    wk_d = nc.dram_tensor("wkb", [128, KC * 128], BF16, kind="ExternalInput")
    wv_d = nc.dram_tensor("wvb", [128, KC * 128], BF16, kind="ExternalInput")
    wout_d = nc.dram_tensor("woutb", [64, 2 * INNER], BF16, kind="ExternalInput")
    out_d = nc.dram_tensor("outp", [128, KC, NQ], F32, kind="ExternalOutput")

    with tile.TileContext(nc) as tc:
        with (
            tc.tile_pool(name="const", bufs=1) as cpool,
            tc.tile_pool(name="big", bufs=1) as big,
            tc.tile_pool(name="ctx", bufs=4) as ctxpool,
            tc.tile_pool(name="pt", bufs=3) as ptpool,
            tc.tile_pool(name="fin", bufs=2) as fin,
            tc.tile_pool(name="ps", bufs=2, space="PSUM") as pps,
            tc.tile_pool(name="kv", bufs=2, space="PSUM") as pkv,
            tc.tile_pool(name="av", bufs=1, space="PSUM") as pav,
        ):
            # ---- small inputs ----
            msk_u8 = cpool.tile([128, NJT], U8)
            nc.sync.dma_start(out=msk_u8[:], in_=msk_d[:, :])
            wq_bf = cpool.tile([128, KC, 128], BF16)
            wk_bf = cpool.tile([128, KC, 128], BF16)
            wv_bf = cpool.tile([128, KC, 128], BF16)
            xt_bf = cpool.tile([128, KC, NQ], BF16)
            wout_bf = cpool.tile([64, 2, INNER], BF16)

            msk_f = cpool.tile([128, NJT], F32)
            nc.vector.tensor_copy(msk_f[:], msk_u8[:])
            # ACT exp bias per j: PT_SHIFT where kept, -inf-ish where masked
            abias = cpool.tile([128, NJT], F32)
            nc.vector.tensor_scalar(abias[:], msk_f[:], MASK_BIG,
                                    PT_SHIFT - MASK_BIG, ALU.mult, ALU.add)
            # DVE Schraudolph bias per j (clamped to u8=0 where masked)
            dbias = cpool.tile([128, NJT], F32)
            nc.vector.tensor_scalar(dbias[:], msk_f[:], 1000.0,
                                    EXP_B - 1000.0, ALU.mult, ALU.add)

            ones_sb = cpool.tile([65, 65], F32)
            nc.vector.memset(ones_sb[64:65, :], 1.0)

            # HAM warm-up: stream free matmuls so the activity monitor
            # unthrottles the PE clock before real work lands.
            warm_ps = pps.tile([65, 65], F32, tag="ps", name="warm_ps")
            for w in range(40):
                nc.tensor.matmul(warm_ps[:], ones_sb[64:65, :],
                                 ones_sb[64:65, :], start=True, stop=True)

            # ---- persistent K^T (bf16) and V (fp8) ----
            kt_bf = big.tile([128, NCTX], BF16)
            # v_both[j, jt, 160]: per head h cols [80h..80h+63]=v,
            # col 80h+64 = 1.0 (denominator), rest zero pad.
            v_both = big.tile([128, NJT, 160], FP8)
            nc.vector.memset(v_both[:, :, 65:80], 0.0)
            nc.vector.memset(v_both[:, :, 145:160], 0.0)
            nc.vector.memset(v_both[:, :, 64:65], 1.0)
            nc.vector.memset(v_both[:, :, 144:145], 1.0)

            qt_holder = []
            psum_av = [pav.tile([80, NQ], F32, tag=f"av{h}", name=f"psum_av{h}")
                       for h in range(2)]

            def ctx_dma(j0, width):
                ctxb = ctxpool.tile([128, KC, width], BF16, tag="ctx",
                                    name=f"ctx_{j0}")
                nc.sync.dma_start(
                    out=ctxb[:],
                    in_=ctx_d.ap()[:, j0:j0 + width].rearrange(
                        "(k p) j -> p k j", p=128))
                return ctxb

            def kv_units(ctxb, j0, width):
                """KV work units for a chunk, interleavable with attention."""
                def kt_unit(c0, w):
                    psum_kt = pkv.tile([128, 512], F32, tag="kv",
                                       name=f"pkt_{j0}_{c0}")
                    for k in range(KC):
                        nc.tensor.matmul(
                            psum_kt[:, 0:w], wk_bf[:, k, :],
                            ctxb[:, k, c0:c0 + w],
                            start=(k == 0), stop=(k == KC - 1))
                    nc.scalar.copy(
                        kt_bf[:, j0 + c0:j0 + c0 + w], psum_kt[:, 0:w])

                def v_unit(c0, w):
                    jt0 = (j0 + c0) // 128
                    nt = w // 128
                    psum_vs = pkv.tile([128, 4, 128], F32, tag="kv",
                                       name=f"pv_{jt0}")
                    for t in range(nt):
                        for k in range(KC):
                            nc.tensor.matmul(
                                psum_vs[:, t, :],
                                ctxb[:, k, c0 + t * 128:c0 + (t + 1) * 128],
                                wv_bf[:, k, :],
                                start=(k == 0), stop=(k == KC - 1))
                    # one strided cast: psum [128,nt,2,64] -> v_both[:,
                    # jt0:jt0+nt, {0-63, 80-143}]
                    nc.vector.tensor_copy(
                        v_both[:, jt0:jt0 + nt, :].rearrange(
                            "p t (h d) -> p t h d", h=2)[:, :, :, 0:64],
                        psum_vs[:, 0:nt, :].rearrange(
                            "p t (h d) -> p t h d", h=2))

                units = []
                for c0 in range(0, width, 512):
                    w = min(512, width - c0)
                    units.append(lambda c0=c0, w=w: kt_unit(c0, w))
                    units.append(lambda c0=c0, w=w: v_unit(c0, w))
                return units

            def attn_pair(p):
                """Scores+exp for tiles 2p, 2p+1 then fp8-DR AV on the pair."""
                pt_pair = ptpool.tile([128, 2, 1024], U8, tag="pt",
                                      name=f"pt_{p}")
                for s in range(2):
                    t = 2 * p + s
                    psum_s = pps.tile([128, 2 * NQ], F32, tag="ps",
                                      name=f"ps_s{t}")
                    for h in range(2):
                        nc.tensor.matmul(psum_s[:, h * NQ:(h + 1) * NQ],
                                         kt_bf[h * 64:(h + 1) * 64,
                                               t * 128:(t + 1) * 128],
                                         qt_holder[0][h * 64:(h + 1) * 64, :],
                                         start=True, stop=True)
                    # late tiles: per-head exp halves the scores->exp->AV
                    # chain latency (cadence there is latency-bound)
                    parts = ((0, 512), (512, 1024)) if t >= 112 else ((0, 1024),)
                    for c0, c1 in parts:
                        if t % 16 in (1, 3, 5, 7, 9, 11, 13):
                            # Schraudolph: u8 = round(A*psum + B[j]); uint8
                            # saturation clamps negatives to 0; B keeps the
                            # max under 120 (fp8 inf/nan zone) for any score.
                            nc.vector.tensor_scalar(pt_pair[:, s, c0:c1],
                                                    psum_s[:, c0:c1],
                                                    EXP_A, dbias[:, t:t + 1],
                                                    ALU.mult, ALU.add)
                        else:
                            nc.scalar.activation(
                                pt_pair[:, s, c0:c1].bitcast(FP8),
                                psum_s[:, c0:c1], AF.Exp,
                                bias=abias[:, t:t + 1], scale=SCALE)
                return pt_pair

            def attn_av(p, pt_pair):
                ptf = pt_pair[:].bitcast(FP8)
                for h in range(2):
                    nc.tensor.matmul(
                        psum_av[h][:],
                        v_both[:, 2 * p:2 * p + 2, 80 * h:80 * h + 80],
                        ptf[:, :, h * 512:(h + 1) * 512],
                        start=(p == 0), stop=(p == NPAIR - 1),
                        perf_mode=DR, skip_group_check=True)

            def emit_qt():
                psum_q = pps.tile([128, NQ], F32, tag="ps", name="psum_q")
                for k in range(KC):
                    nc.tensor.matmul(psum_q[:], wq_bf[:, k, :], xt_bf[:, k, :],
                                     start=(k == 0), stop=(k == KC - 1))
                qt_bf = cpool.tile([128, NQ], BF16, name="qt_bf")
                nc.vector.tensor_copy(qt_bf[:], psum_q[:])
                qt_holder.append(qt_bf)

            # DMA order: first ctx piece + kv weights first, then q/x, rest.
            warm = [(0, 256), (256, 256)] + [(j0, 512)
                                             for j0 in range(512, JCH, 512)]
            # taper the tail chunks so only the final 512-piece's pairs
            # lack interleavable KV work (keeps the PE/exp pipeline full)
            rest = [(c * JCH, JCH) for c in range(1, NCH - 1)]
            t0 = (NCH - 1) * JCH
            rest += [(t0, 1024), (t0 + 1024, 512), (t0 + 1536, 512)]
            pieces = warm + rest

            for dst, srcw in ((wk_bf, wk_d), (wv_bf, wv_d)):
                nc.sync.dma_start(
                    out=dst[:], in_=srcw.ap().rearrange("p (k n) -> p k n", k=KC))
            handles = [ctx_dma(*pieces[0]), ctx_dma(*pieces[1])]
            for dst, srcw in ((wq_bf, wq_d), (xt_bf, xt_d)):
                nc.sync.dma_start(
                    out=dst[:], in_=srcw.ap().rearrange("p (k n) -> p k n", k=KC))

            def ensure_dma(idx):
                while len(handles) <= min(idx, len(pieces) - 1):
                    handles.append(ctx_dma(*pieces[len(handles)]))

            ensure_dma(2)
            kt0, v0 = kv_units(handles[0], *pieces[0])
            kt0()
            emit_qt()
            v0()
            for i in range(len(pieces)):
                if i == 2:
                    nc.sync.dma_start(
                        out=wout_bf[:],
                        in_=wout_d.ap().rearrange("p (h n) -> p h n", h=2))
                j0, width = pieces[i]
                pairs = list(range(j0 // 256, (j0 + width) // 256))
                units = []
                if i + 1 < len(pieces):
                    ensure_dma(i + 2)
                    units = kv_units(handles[i + 1], *pieces[i + 1])
                per = (len(units) + len(pairs) - 1) // max(len(pairs), 1)
                ui = 0
                for p in pairs:
                    ptp = attn_pair(p)
                    for _ in range(per):
                        if ui < len(units):
                            units[ui]()
                            ui += 1
                    attn_av(p, ptp)
                while ui < len(units):
                    units[ui]()
                    ui += 1

            # ---- normalize by softmax denominator (row 64 of psum_av) ----
            # broadcast den across partitions via K=1 fp32r matmul, then
            # reciprocal + multiply per head; no DMA roundtrip.
            avn = fin.tile([64, 2, NQ], BF16, tag="avn")
            l2 = fin.tile([65, 2 * NQ], mybir.dt.float32r, tag="l2")
            for h in range(2):
                nc.vector.tensor_copy(l2[64:65, h * NQ:(h + 1) * NQ],
                                      psum_av[h][64:65, :])
                psum_lb = pps.tile([65, NQ], F32, tag="ps", name=f"plb_{h}")
                nc.tensor.matmul(psum_lb[:],
                                 ones_sb[64:65, :].bitcast(mybir.dt.float32r),
                                 l2[64:65, h * NQ:(h + 1) * NQ],
                                 start=True, stop=True)
                linvb = fin.tile([64, NQ], F32, tag="linvb", name=f"lb_{h}")
                nc.vector.reciprocal_approx_fast(out=linvb[:],
                                                 in_=psum_lb[0:64, :])
                nc.vector.tensor_tensor(avn[:, h, :], psum_av[h][0:64, :],
                                        linvb[:], ALU.mult)

            # ---- partial output projection: P = Wout_slice^T @ avn ----
            out_sb = fin.tile([128, KC, NQ], F32, tag="out")
            for c in range(KC):
                psum_o = pkv.tile([128, NQ], F32, tag="kv", name=f"po_{c}")
                for h in range(2):
                    nc.tensor.matmul(
                        psum_o[:], wout_bf[:, h, c * 128:(c + 1) * 128],
                        avn[:, h, :], start=(h == 0), stop=(h == 1))
                if c % 2 == 0:
                    nc.scalar.copy(out_sb[:, c, :], psum_o[:])
                else:
                    nc.vector.tensor_copy(out_sb[:, c, :], psum_o[:])
            nc.sync.dma_start(out=out_d[:, :, :], in_=out_sb[:])

    nc.compile()
    return nc


_NC = None


def _get_nc():
    global _NC
    if _NC is None:
        _NC = build_nc()
    return _NC


def make_in_maps(x, context, mask, Wq, Wkv, Wout, bout):
    ctxb = [np.ascontiguousarray(context[b].T).astype(ml_dtypes.bfloat16)
            for b in range(B)]
    xtb = [np.ascontiguousarray(x[b].T).astype(ml_dtypes.bfloat16)
           for b in range(B)]
    mskb = [np.ascontiguousarray(
        mask[b].reshape(NJT, 128).T.astype(np.uint8)) for b in range(B)]
    def pack(w):  # [512, n] -> [128, KC*n] with k-chunk minor
        n = w.shape[1]
        return np.ascontiguousarray(
            w.reshape(KC, 128, n).transpose(1, 0, 2).reshape(128, KC * n))
    wk_b = Wkv[:, :INNER].astype(ml_dtypes.bfloat16)
    wv_b = Wkv[:, INNER:].astype(ml_dtypes.bfloat16)
    wq_b = Wq.astype(ml_dtypes.bfloat16)
    in_maps = []
    for core in range(N_CORES):
        b, hg = core // 4, core % 4
        cs = slice(hg * 128, (hg + 1) * 128)
        # wout rows for this core's 2 heads: [64, 2*512] (h-major cols)
        wo = Wout[cs, :]
        woutb = np.concatenate([wo[0:64, :], wo[64:128, :]],
                               axis=1).astype(ml_dtypes.bfloat16)
        in_maps.append({
            "xT": pack(xtb[b]),
            "ctxb": ctxb[b],
            "maskt": mskb[b],
            "wq": pack(wq_b[:, cs]),
            "wkb": pack(wk_b[:, cs]),
            "wvb": pack(wv_b[:, cs]),
            "woutb": np.ascontiguousarray(woutb),
        })
    return in_maps


def kernel(x, context, mask, Wq, Wkv, Wout, bout):
    x = np.asarray(x, dtype=np.float32)
    context = np.asarray(context, dtype=np.float32)
    mask = np.asarray(mask)
    Wq = np.asarray(Wq, dtype=np.float32)
    Wkv = np.asarray(Wkv, dtype=np.float32)
    Wout = np.asarray(Wout, dtype=np.float32)
    bout = np.asarray(bout, dtype=np.float32)

    nc = _get_nc()
    in_maps = make_in_maps(x, context, mask, Wq, Wkv, Wout, bout)
    res = run_bass_kernel_spmd(nc, in_maps, list(range(N_CORES)))
    out = np.empty((B, NQ, INNER), dtype=np.float32)
    for b in range(B):
        acc = np.zeros((INNER, NQ), dtype=np.float32)
        for hg in range(4):
            p = res.results[4 * b + hg]["outp"]  # [128, KC, NQ]
            acc += p.transpose(1, 0, 2).reshape(INNER, NQ)
        out[b] = acc.T + bout
    return out


# revision 31
# speedup vs baseline: 1.1974x; 1.1974x over previous
"""Distributed Bass attention kernel for 8 TRN2 NeuronCores.

Problem: nn_Attention (B=2, NQ=512, NCTX=16384, QDIM=CDIM=512, H=8, D=64).

Sharding: data parallel on batch (2) x tensor parallel on heads (4 groups of
2 heads) = 8 cores. Core i handles batch i//4, heads [2*(i%4), 2*(i%4)+1].
Each core computes its head-slice of the attention output, normalizes it,
multiplies by its slice of Wout to produce a PARTIAL output projection
[out 512, q 512]; the host sums the 4 partials per batch (no on-device
collective) and adds bout.

Device-side layout/perf choices:
  - context is passed pre-transposed AND pre-cast to bf16 ([d_model, seq]),
    halving HBM traffic for the big tensor (16 MiB/core). wk/wv stay bf16
    (fp8 weights cost ~3% rel err - measured, too much).
  - scores are computed transposed (simT[j, i]) in bf16 (contraction d=64),
    so the context mask is a per-partition bias of the exp and the softmax
    denominator falls out of the AV matmul via a ones-column in V.
  - exp runs on BOTH the Scalar engine (ACT exp -> fp8e4) and the Vector
    engine (single-op Schraudolph affine to uint8, relying on the convert's
    round+saturate-at-0; the bit-pattern IS the fp8), split ~9:7 per t%16
    to balance engine busy time.
  - attention weights (pt) and V are fp8e4; pt is scaled 1/16 (softmax
    ratio invariant) so exp never overflows fp8 range.
  - AV runs as fp8 DoubleRow over j-tile pairs (contraction 256/instr,
    2x fewer PE rows than bf16).
  - attention j-tile pairs interleave with the next context chunk's KV
    units so the PE never idles; KT copies ride ACT, V copies ride DVE.
"""
import sys

sys.path.insert(0, '/opt/trn_rl_repo')

import numpy as np
import ml_dtypes

import concourse.bacc as bacc
import concourse.mybir as mybir
import concourse.tile as tile
from concourse.bass_utils import run_bass_kernel_spmd

F32 = mybir.dt.float32
BF16 = mybir.dt.bfloat16
FP8 = mybir.dt.float8e4
U8 = mybir.dt.uint8
AF = mybir.ActivationFunctionType
ALU = mybir.AluOpType
DR = mybir.MatmulPerfMode.DoubleRow

B = 2
NQ = 512          # query tokens (i)
NCTX = 16384      # context tokens (j)
DM = 512          # model dim
HEADS = 8
DH = 64
INNER = 512
N_CORES = 8

KC = 4              # d_model chunks of 128
NJT = NCTX // 128   # 128 j-tiles
NPAIR = NJT // 2    # 64 j-tile pairs
JCH = 2048          # context j-chunk per DMA (1 MiB fp8 source)
NCH = NCTX // JCH

SCALE = DH ** -0.5                       # s_true = psum * SCALE
PT_SHIFT = -2.7726                       # pt = exp(s_true)/16, keeps fp8<240
MASK_BIG = 30000.0
# Schraudolph constants for fp8e4 (ieee e4m3, bias 7, 3 mantissa bits):
#   u8 = clamp(A*psum + B[j], 0, 119); bitpattern u8 == fp8 ~ exp-approx
EXP_A = float(8.0 * np.log2(np.e) * SCALE)
EXP_B = float(56.0 + 8.0 * PT_SHIFT * np.log2(np.e) - 0.44)


def build_nc():
    nc = bacc.Bacc(None, target_bir_lowering=False, debug=False, num_devices=N_CORES)

    xt_d = nc.dram_tensor("xT", [128, KC * NQ], BF16, kind="ExternalInput")
    ctx_d = nc.dram_tensor("ctxb", [DM, NCTX], BF16, kind="ExternalInput")
    msk_d = nc.dram_tensor("maskt", [128, NJT], U8, kind="ExternalInput")
    wq_d = nc.dram_tensor("wq", [128, KC * 128], BF16, kind="ExternalInput")
    wk_d = nc.dram_tensor("wkb", [128, KC * 128], BF16, kind="ExternalInput")
    wv_d = nc.dram_tensor("wvb", [128, KC * 128], BF16, kind="ExternalInput")
    wout_d = nc.dram_tensor("woutb", [64, 2 * INNER], BF16, kind="ExternalInput")
    out_d = nc.dram_tensor("outp", [128, KC, NQ], F32, kind="ExternalOutput")

    with tile.TileContext(nc) as tc:
        with (
            tc.tile_pool(name="const", bufs=1) as cpool,
            tc.tile_pool(name="big", bufs=1) as big,
            tc.tile_pool(name="ctx", bufs=4) as ctxpool,
            tc.tile_pool(name="pt", bufs=3) as ptpool,
            tc.tile_pool(name="fin", bufs=2) as fin,
            tc.tile_pool(name="ps", bufs=2, space="PSUM") as pps,
            tc.tile_pool(name="kv", bufs=2, space="PSUM") as pkv,
            tc.tile_pool(name="av", bufs=1, space="PSUM") as pav,
        ):
            # ---- small inputs ----
            msk_u8 = cpool.tile([128, NJT], U8)
            nc.sync.dma_start(out=msk_u8[:], in_=msk_d[:, :])
            wq_bf = cpool.tile([128, KC, 128], BF16)
            wk_bf = cpool.tile([128, KC, 128], BF16)
            wv_bf = cpool.tile([128, KC, 128], BF16)
            xt_bf = cpool.tile([128, KC, NQ], BF16)
            wout_bf = cpool.tile([64, 2, INNER], BF16)

            msk_f = cpool.tile([128, NJT], F32)
            nc.vector.tensor_copy(msk_f[:], msk_u8[:])
            # ACT exp bias per j: PT_SHIFT where kept, -inf-ish where masked
            abias = cpool.tile([128, NJT], F32)
            nc.vector.tensor_scalar(abias[:], msk_f[:], MASK_BIG,
                                    PT_SHIFT - MASK_BIG, ALU.mult, ALU.add)
            # DVE Schraudolph bias per j (clamped to u8=0 where masked)
            dbias = cpool.tile([128, NJT], F32)
            nc.vector.tensor_scalar(dbias[:], msk_f[:], 1000.0,
                                    EXP_B - 1000.0, ALU.mult, ALU.add)

            ones_sb = cpool.tile([65, 65], F32)
            nc.vector.memset(ones_sb[64:65, :], 1.0)

            # HAM warm-up: stream free matmuls so the activity monitor
            # unthrottles the PE clock before real work lands.
            warm_ps = pps.tile([65, 65], F32, tag="ps", name="warm_ps")
            for w in range(40):
                nc.tensor.matmul(warm_ps[:], ones_sb[64:65, :],
                                 ones_sb[64:65, :], start=True, stop=True)

            # ---- persistent K^T (bf16) and V (fp8) ----
            kt_bf = big.tile([128, NCTX], BF16)
            # v_both[j, jt, 160]: per head h cols [80h..80h+63]=v,
            # col 80h+64 = 1.0 (denominator), rest zero pad.
            v_both = big.tile([128, NJT, 160], FP8)
            nc.gpsimd.memset(v_both[:, :, 65:80], 0.0)
            nc.gpsimd.memset(v_both[:, :, 145:160], 0.0)
            nc.gpsimd.memset(v_both[:, :, 64:65], 1.0)
            nc.gpsimd.memset(v_both[:, :, 144:145], 1.0)

            qt_holder = []
            psum_av = [pav.tile([80, NQ], F32, tag=f"av{h}", name=f"psum_av{h}")
                       for h in range(2)]

            def ctx_dma(j0, width):
                ctxb = ctxpool.tile([128, KC, width], BF16, tag="ctx",
                                    name=f"ctx_{j0}")
                nc.sync.dma_start(
                    out=ctxb[:],
                    in_=ctx_d.ap()[:, j0:j0 + width].rearrange(
                        "(k p) j -> p k j", p=128))
                return ctxb

            def kv_units(ctxb, j0, width):
                """KV work units for a chunk, interleavable with attention."""
                def kt_unit(c0, w):
                    psum_kt = pkv.tile([128, 512], F32, tag="kv",
                                       name=f"pkt_{j0}_{c0}")
                    for k in range(KC):
                        nc.tensor.matmul(
                            psum_kt[:, 0:w], wk_bf[:, k, :],
                            ctxb[:, k, c0:c0 + w],
                            start=(k == 0), stop=(k == KC - 1))
                    nc.scalar.copy(
                        kt_bf[:, j0 + c0:j0 + c0 + w], psum_kt[:, 0:w])

                def v_unit(c0, w):
                    jt0 = (j0 + c0) // 128
                    nt = w // 128
                    psum_vs = pkv.tile([128, 4, 128], F32, tag="kv",
                                       name=f"pv_{jt0}")
                    for t in range(nt):
                        for k in range(KC):
                            nc.tensor.matmul(
                                psum_vs[:, t, :],
                                ctxb[:, k, c0 + t * 128:c0 + (t + 1) * 128],
                                wv_bf[:, k, :],
                                start=(k == 0), stop=(k == KC - 1))
                    # one strided cast: psum [128,nt,2,64] -> v_both[:,
                    # jt0:jt0+nt, {0-63, 80-143}]
                    nc.vector.tensor_copy(
                        v_both[:, jt0:jt0 + nt, :].rearrange(
                            "p t (h d) -> p t h d", h=2)[:, :, :, 0:64],
                        psum_vs[:, 0:nt, :].rearrange(
                            "p t (h d) -> p t h d", h=2))

                units = []
                for c0 in range(0, width, 512):
                    w = min(512, width - c0)
                    units.append(lambda c0=c0, w=w: kt_unit(c0, w))
                    units.append(lambda c0=c0, w=w: v_unit(c0, w))
                return units

            def attn_pair(p):
                """Scores+exp for tiles 2p, 2p+1 then fp8-DR AV on the pair."""
                pt_pair = ptpool.tile([128, 2, 1024], U8, tag="pt",
                                      name=f"pt_{p}")
                for s in range(2):
                    t = 2 * p + s
                    psum_s = pps.tile([128, 2 * NQ], F32, tag="ps",
                                      name=f"ps_s{t}")
                    for h in range(2):
                        nc.tensor.matmul(psum_s[:, h * NQ:(h + 1) * NQ],
                                         kt_bf[h * 64:(h + 1) * 64,
                                               t * 128:(t + 1) * 128],
                                         qt_holder[0][h * 64:(h + 1) * 64, :],
                                         start=True, stop=True)
                    # late tiles: per-head exp halves the scores->exp->AV
                    # chain latency (cadence there is latency-bound)
                    parts = ((0, 512), (512, 1024)) if t >= 112 else ((0, 1024),)
                    for c0, c1 in parts:
                        if t % 16 in (1, 3, 5, 7, 9, 11, 13):
                            # Schraudolph: u8 = round(A*psum + B[j]); uint8
                            # saturation clamps negatives to 0; B keeps the
                            # max under 120 (fp8 inf/nan zone) for any score.
                            nc.vector.tensor_scalar(pt_pair[:, s, c0:c1],
                                                    psum_s[:, c0:c1],
                                                    EXP_A, dbias[:, t:t + 1],
                                                    ALU.mult, ALU.add)
                        else:
                            nc.scalar.activation(
                                pt_pair[:, s, c0:c1].bitcast(FP8),
                                psum_s[:, c0:c1], AF.Exp,
                                bias=abias[:, t:t + 1], scale=SCALE)
                return pt_pair

            def attn_av(p, pt_pair):
                ptf = pt_pair[:].bitcast(FP8)
                for h in range(2):
                    nc.tensor.matmul(
                        psum_av[h][:],
                        v_both[:, 2 * p:2 * p + 2, 80 * h:80 * h + 80],
                        ptf[:, :, h * 512:(h + 1) * 512],
                        start=(p == 0), stop=(p == NPAIR - 1),
                        perf_mode=DR, skip_group_check=True)

            def emit_qt():
                psum_q = pps.tile([128, NQ], F32, tag="ps", name="psum_q")
                for k in range(KC):
                    nc.tensor.matmul(psum_q[:], wq_bf[:, k, :], xt_bf[:, k, :],
                                     start=(k == 0), stop=(k == KC - 1))
                qt_bf = cpool.tile([128, NQ], BF16, name="qt_bf")
                nc.vector.tensor_copy(qt_bf[:], psum_q[:])
                qt_holder.append(qt_bf)

            # DMA order: first ctx piece + kv weights first, then q/x, rest.
            warm = [(0, 256), (256, 256)] + [(j0, 512)
                                             for j0 in range(512, JCH, 512)]
            # taper the tail chunks so only the final 512-piece's pairs
            # lack interleavable KV work (keeps the PE/exp pipeline full)
            rest = [(c * JCH, JCH) for c in range(1, NCH - 1)]
            t0 = (NCH - 1) * JCH
            rest += [(t0, 1024), (t0 + 1024, 512), (t0 + 1536, 512)]
            pieces = warm + rest

            for dst, srcw in ((wk_bf, wk_d), (wv_bf, wv_d)):
                nc.sync.dma_start(
                    out=dst[:], in_=srcw.ap().rearrange("p (k n) -> p k n", k=KC))
            handles = [ctx_dma(*pieces[0]), ctx_dma(*pieces[1])]
            for dst, srcw in ((wq_bf, wq_d), (xt_bf, xt_d)):
                nc.sync.dma_start(
                    out=dst[:], in_=srcw.ap().rearrange("p (k n) -> p k n", k=KC))

            def ensure_dma(idx):
                while len(handles) <= min(idx, len(pieces) - 1):
                    handles.append(ctx_dma(*pieces[len(handles)]))

            ensure_dma(2)
            kt0, v0 = kv_units(handles[0], *pieces[0])
            kt0()
            emit_qt()
            v0()
            for i in range(len(pieces)):
                if i == 2:
                    nc.sync.dma_start(
                        out=wout_bf[:],
                        in_=wout_d.ap().rearrange("p (h n) -> p h n", h=2))
                j0, width = pieces[i]
                pairs = list(range(j0 // 256, (j0 + width) // 256))
                units = []
                if i + 1 < len(pieces):
                    ensure_dma(i + 2)
                    units = kv_units(handles[i + 1], *pieces[i + 1])
                per = (len(units) + len(pairs) - 1) // max(len(pairs), 1)
                ui = 0
                for p in pairs:
                    ptp = attn_pair(p)
                    for _ in range(per):
                        if ui < len(units):
                            units[ui]()
                            ui += 1
                    attn_av(p, ptp)
                while ui < len(units):
                    units[ui]()
                    ui += 1

            # ---- normalize by softmax denominator (row 64 of psum_av) ----
            # broadcast den across partitions via K=1 fp32r matmul, then
            # reciprocal + multiply per head; no DMA roundtrip.
            avn = fin.tile([64, 2, NQ], BF16, tag="avn")
            l2 = fin.tile([65, 2 * NQ], mybir.dt.float32r, tag="l2")
            for h in range(2):
                nc.vector.tensor_copy(l2[64:65, h * NQ:(h + 1) * NQ],
                                      psum_av[h][64:65, :])
                psum_lb = pps.tile([65, NQ], F32, tag="ps", name=f"plb_{h}")
                nc.tensor.matmul(psum_lb[:],
                                 ones_sb[64:65, :].bitcast(mybir.dt.float32r),
                                 l2[64:65, h * NQ:(h + 1) * NQ],
                                 start=True, stop=True)
                linvb = fin.tile([64, NQ], F32, tag="linvb", name=f"lb_{h}")
                nc.vector.reciprocal_approx_fast(out=linvb[:],
                                                 in_=psum_lb[0:64, :])
                nc.vector.tensor_tensor(avn[:, h, :], psum_av[h][0:64, :],
                                        linvb[:], ALU.mult)

            # ---- partial output projection: P = Wout_slice^T @ avn ----
            out_sb = fin.tile([128, KC, NQ], F32, tag="out")
            for c in range(KC):
                psum_o = pkv.tile([128, NQ], F32, tag="kv", name=f"po_{c}")
                for h in range(2):
                    nc.tensor.matmul(
                        psum_o[:], wout_bf[:, h, c * 128:(c + 1) * 128],
                        avn[:, h, :], start=(h == 0), stop=(h == 1))
                if c % 2 == 0:
                    nc.scalar.copy(out_sb[:, c, :], psum_o[:])
                else:
                    nc.vector.tensor_copy(out_sb[:, c, :], psum_o[:])
            nc.sync.dma_start(out=out_d[:, :, :], in_=out_sb[:])

    nc.compile()
    return nc


_NC = None


def _get_nc():
    global _NC
    if _NC is None:
        _NC = build_nc()
    return _NC


def make_in_maps(x, context, mask, Wq, Wkv, Wout, bout):
    ctxb = [np.ascontiguousarray(context[b].T).astype(ml_dtypes.bfloat16)
            for b in range(B)]
    xtb = [np.ascontiguousarray(x[b].T).astype(ml_dtypes.bfloat16)
           for b in range(B)]
    mskb = [np.ascontiguousarray(
        mask[b].reshape(NJT, 128).T.astype(np.uint8)) for b in range(B)]
    def pack(w):  # [512, n] -> [128, KC*n] with k-chunk minor
        n = w.shape[1]
        return np.ascontiguousarray(
            w.reshape(KC, 128, n).transpose(1, 0, 2).reshape(128, KC * n))
    wk_b = Wkv[:, :INNER].astype(ml_dtypes.bfloat16)
    wv_b = Wkv[:, INNER:].astype(ml_dtypes.bfloat16)
    wq_b = Wq.astype(ml_dtypes.bfloat16)
    in_maps = []
    for core in range(N_CORES):
        b, hg = core // 4, core % 4
        cs = slice(hg * 128, (hg + 1) * 128)
        # wout rows for this core's 2 heads: [64, 2*512] (h-major cols)
        wo = Wout[cs, :]
        woutb = np.concatenate([wo[0:64, :], wo[64:128, :]],
                               axis=1).astype(ml_dtypes.bfloat16)
        in_maps.append({
            "xT": pack(xtb[b]),
            "ctxb": ctxb[b],
            "maskt": mskb[b],
            "wq": pack(wq_b[:, cs]),
            "wkb": pack(wk_b[:, cs]),
            "wvb": pack(wv_b[:, cs]),
            "woutb": np.ascontiguousarray(woutb),
        })
    return in_maps


def kernel(x, context, mask, Wq, Wkv, Wout, bout):
    x = np.asarray(x, dtype=np.float32)
    context = np.asarray(context, dtype=np.float32)
    mask = np.asarray(mask)
    Wq = np.asarray(Wq, dtype=np.float32)
    Wkv = np.asarray(Wkv, dtype=np.float32)
    Wout = np.asarray(Wout, dtype=np.float32)
    bout = np.asarray(bout, dtype=np.float32)

    nc = _get_nc()
    in_maps = make_in_maps(x, context, mask, Wq, Wkv, Wout, bout)
    res = run_bass_kernel_spmd(nc, in_maps, list(range(N_CORES)))
    out = np.empty((B, NQ, INNER), dtype=np.float32)
    for b in range(B):
        acc = np.zeros((INNER, NQ), dtype=np.float32)
        for hg in range(4):
            p = res.results[4 * b + hg]["outp"]  # [128, KC, NQ]
            acc += p.transpose(1, 0, 2).reshape(INNER, NQ)
        out[b] = acc.T + bout
    return out
